# revision 11
# baseline (speedup 1.0000x reference)
"""Trainium2 Bass kernel for the ChemotaxisPINN loss.

Computes loss = mean_col((u_t - D*u_xx + chi*(u_x*S'(x) + u*S''(x)))^2)
             + mean_ic((u - ic(x))^2) + mean_bc(u_x(0,t)^2) + mean_bc(u_x(1,t)^2)
for a 5-layer SiLU MLP u(x,t), via forward-mode AD on device.

Strategy: pure data parallel over 8 NeuronCores. Each core gets a shard of
collocation/ic/bc points (17134 valid, padded to 17408 = 34 chunks x 512).
On device, per 512-point chunk, the MLP is evaluated with 4 propagated
streams (value, d/dx, d/dt, d2/dx2) in a feature-major layout
[128 features x points]; per-point residual coefficients are precomputed on
the host and folded so that loss = sum over all points of r^2 with
r = A*u_t + B*u_xx + C1*u_x + C2*u - T.  Each core returns a partial sum;
the host adds the 8 partials.

Perf layout (v2): bf16 matmuls/activations (fp32 PSUM accum), a fused
custom-DVE op for the silu'/silu'' chains, the three tangent preactivations
packed in one 3-bank PSUM tile evacuated by a single scalar ACT, and the
four scalar outputs packed in one 2-bank PSUM tile evacuated by one ACT.
Elementwise work is spread over scalar/vector/gpsimd.

silu-family identities (z = preactivation + bias, s = sigmoid(z)):
  a   = z * s                 = silu(z)
  sp  = a + s*(1-a)           = silu'(z)   (one fused DVE op)
  spp = s + sp*(1-2s)         = silu''(z)  (one fused DVE op)
"""

import numpy as np
import ml_dtypes

import orjson
import concourse.bass as bass
import concourse.tile as tile
from concourse import mybir
from concourse.bass_utils import run_bass_kernel_spmd

F32 = mybir.dt.float32
BF16 = mybir.dt.bfloat16
AL = mybir.AluOpType
AF = mybir.ActivationFunctionType

N_CORES = 8
H = 128
N_COL, N_IC, N_BC = 131072, 2000, 2000
COLC = N_COL // N_CORES     # 16384
ICC = N_IC // N_CORES       # 250
BCC = N_BC // N_CORES       # 250
NVALID = COLC + ICC + 2 * BCC   # 17134
CH = 512                     # points per chunk
NCHUNK = (NVALID + CH - 1) // CH  # 34
NPTS = NCHUNK * CH           # 17408
F2 = NPTS // 128             # 136 (phase-2 free dim)

# ---------------------------------------------------------------------------
# BIR fix: this walrus build accepts at most ONE sem wait per instruction,
# while Tile attaches several.  Split extras onto single-wait NoOps.
# ---------------------------------------------------------------------------
_orig_to_json_bytes = bass.Bass.to_json_bytes


def _split_multiwait(m):
    for fn in m.get("functions", []):
        for blk in fn.get("blocks", []):
            insts = blk.get("instructions", [])
            out = []
            changed = False
            ctr = 0
            for inst in insts:
                si = inst.get("sync_info")
                waits = (si or {}).get("on_wait") or []
                if len(waits) > 1:
                    changed = True
                    for w in waits[:-1]:
                        ctr += 1
                        out.append({
                            "engine": inst["engine"],
                            "ins": [],
                            "outs": [],
                            "name": f"I-mws-{ctr}-{inst.get('name', '')}",
                            "opcode": "NoOp",
                            "sync_info": {"on_wait": [w], "on_update": []},
                            "debug": inst.get("debug", 0),
                        })
                    si["on_wait"] = waits[-1:]
                out.append(inst)
            if changed:
                blk["instructions"] = out
    return m


def _patched_to_json_bytes(self):
    return orjson.dumps(_split_multiwait(orjson.loads(_orig_to_json_bytes(self))))


bass.Bass.to_json_bytes = _patched_to_json_bytes


# ---------------------------------------------------------------------------
# Device program
# ---------------------------------------------------------------------------
def build_program():
    nc = bass.Bass("TRN2", target_bir_lowering=False, debug=False)

    xt_in = nc.declare_dram_parameter("xt", [2, NPTS], BF16, isOutput=False)
    coef_in = nc.declare_dram_parameter("coef", [5, NPTS], F32, isOutput=False)
    w0_in = nc.declare_dram_parameter("w0", [2, H], BF16, isOutput=False)
    w123_in = nc.declare_dram_parameter("w123", [3, H, H], BF16, isOutput=False)
    w4_in = nc.declare_dram_parameter("w4", [H, 1], BF16, isOutput=False)
    pc_in = nc.declare_dram_parameter("pc", [H, 8], F32, isOutput=False)
    part_out = nc.declare_dram_parameter("partial", [1, 1], F32, isOutput=True)

    # DRAM staging for per-point u, ux, ut, uxx (stream-major, chunk rows)
    stage = nc.dram_tensor("stage", [4, NCHUNK, CH], F32)

    with tile.TileContext(nc) as tc:
        with (
            tc.tile_pool(name="consts", bufs=1) as cn,
            tc.tile_pool(name="sb", bufs=3) as sb,
            tc.tile_pool(name="out1", bufs=3) as out1,
            tc.tile_pool(name="ph2", bufs=1) as ph2,
            tc.tile_pool(name="psA", bufs=2, space="PSUM") as psA,
            tc.tile_pool(name="psB", bufs=1, space="PSUM") as psB,
            tc.tile_pool(name="psu", bufs=1, space="PSUM") as psu,
        ):
            # ---- constants ----
            w0_sb = cn.tile([2, H], BF16)
            nc.sync.dma_start(w0_sb[:], w0_in[:])
            w123_sb = cn.tile([H, 3, H], BF16)
            for l in range(3):
                nc.sync.dma_start(w123_sb[:, l, :], w123_in[l])
            w4_sb = cn.tile([H, 1], BF16)
            nc.sync.dma_start(w4_sb[:], w4_in[:])
            pc_sb = cn.tile([H, 8], F32)
            nc.sync.dma_start(pc_sb[:], pc_in[:])
            ones_sb = cn.tile([H, 1], F32)
            nc.vector.memset(ones_sb[:], 1.0)
            # pc columns: 0..3 = b0..b3, 4 = W0[0], 5 = W0[1], 6 = W0[0]^2
            b_ap = [pc_sb[:, i:i + 1] for i in range(4)]
            w0x_ap = pc_sb[:, 4:5]
            w0t_ap = pc_sb[:, 5:6]
            w0x2_ap = pc_sb[:, 6:7]

            # ---- phase 1: per-chunk MLP + tangents ----
            for c in range(NCHUNK):
                xt_sb = sb.tile([2, CH], BF16, tag="xt")
                nc.sync.dma_start(xt_sb[:], xt_in[:, c * CH:(c + 1) * CH])

                # L0: z0 = W0.T @ [x;t]  (K=2)
                z0 = psA.tile([H, CH], F32, tag="z")
                nc.tensor.matmul(z0[:], w0_sb[:], xt_sb[:], start=True, stop=True)

                s_t = sb.tile([H, CH], BF16, tag="s")
                nc.scalar.activation(s_t[:], z0[:], AF.Sigmoid, bias=b_ap[0])
                a_t = sb.tile([H, CH], BF16, tag="a")
                nc.vector.scalar_tensor_tensor(a_t[:], z0[:], b_ap[0], s_t[:], AL.add, AL.mult)
                t_t = sb.tile([H, CH], BF16, tag="t")
                nc.vector.scalar_tensor_tensor(t_t[:], a_t[:], -1.0, s_t[:], AL.add, AL.mult)
                sp_t = sb.tile([H, CH], BF16, tag="sp")
                nc.vector.tensor_tensor(sp_t[:], a_t[:], t_t[:], AL.subtract)
                q_t = sb.tile([H, CH], BF16, tag="q")
                nc.vector.scalar_tensor_tensor(q_t[:], s_t[:], -0.5, sp_t[:], AL.add, AL.mult)
                spp_t = sb.tile([H, CH], BF16, tag="spp")
                nc.vector.scalar_tensor_tensor(spp_t[:], q_t[:], -2.0, s_t[:], AL.mult, AL.add)

                # L0 tangent seeds (per-partition constants)
                ax_t = sb.tile([H, CH], BF16, tag="ax")
                nc.scalar.activation(ax_t[:], sp_t[:], AF.Identity, scale=w0x_ap)
                at_t = sb.tile([H, CH], BF16, tag="at")
                nc.gpsimd.tensor_scalar(at_t[:], sp_t[:], w0t_ap, None, AL.mult)
                q3_t = sb.tile([H, CH], BF16, tag="q3")
                nc.gpsimd.tensor_scalar(q3_t[:], spp_t[:], w0x2_ap, None, AL.mult)
                q2_t = None  # L0 xx-stream has a single component

                # hidden layers 1..3
                for l in range(3):
                    W = w123_sb[:, l, :]
                    zv = psA.tile([H, CH], F32, tag="z")
                    nc.tensor.matmul(zv[:], W, a_t[:], start=True, stop=True)
                    zt3 = psB.tile([H, 3, CH], F32, tag="zt3")
                    nc.tensor.matmul(zt3[:, 0, :], W, ax_t[:], start=True, stop=True)
                    nc.tensor.matmul(zt3[:, 1, :], W, at_t[:], start=True, stop=True)
                    if q2_t is None:
                        nc.tensor.matmul(zt3[:, 2, :], W, q3_t[:], start=True, stop=True)
                    else:
                        nc.tensor.matmul(zt3[:, 2, :], W, q3_t[:], start=True, stop=False)
                        nc.tensor.matmul(zt3[:, 2, :], W, q2_t[:], start=False, stop=True)

                    b = b_ap[l + 1]
                    s_t = sb.tile([H, CH], BF16, tag="s")
                    nc.scalar.activation(s_t[:], zv[:], AF.Sigmoid, bias=b)
                    a_t = sb.tile([H, CH], BF16, tag="a")
                    nc.vector.scalar_tensor_tensor(a_t[:], zv[:], b, s_t[:], AL.add, AL.mult)
                    t_t = sb.tile([H, CH], BF16, tag="t")
                    nc.vector.scalar_tensor_tensor(t_t[:], a_t[:], -1.0, s_t[:], AL.add, AL.mult)
                    sp_t = sb.tile([H, CH], BF16, tag="sp")
                    nc.vector.tensor_tensor(sp_t[:], a_t[:], t_t[:], AL.subtract)
                    q_t = sb.tile([H, CH], BF16, tag="q")
                    nc.vector.scalar_tensor_tensor(q_t[:], s_t[:], -0.5, sp_t[:], AL.add, AL.mult)
                    spp_t = sb.tile([H, CH], BF16, tag="spp")
                    nc.vector.scalar_tensor_tensor(spp_t[:], q_t[:], -2.0, s_t[:], AL.mult, AL.add)

                    # single evacuation of the 3 tangent preactivations
                    uxtc = sb.tile([H, 3, CH], BF16, tag="uxtc")
                    nc.scalar.activation(uxtc[:], zt3[:], AF.Copy)
                    uxc = uxtc[:, 0, :]
                    utc = uxtc[:, 1, :]
                    uxxc = uxtc[:, 2, :]
                    zx2_t = sb.tile([H, CH], BF16, tag="zx2")
                    nc.scalar.activation(zx2_t[:], uxc, AF.Square)

                    nax = sb.tile([H, CH], BF16, tag="ax")
                    nc.vector.tensor_tensor(nax[:], sp_t[:], uxc, AL.mult)
                    nq2 = sb.tile([H, CH], BF16, tag="q2")
                    nc.vector.tensor_tensor(nq2[:], spp_t[:], zx2_t[:], AL.mult)
                    nat = sb.tile([H, CH], BF16, tag="at")
                    nc.gpsimd.tensor_tensor(nat[:], sp_t[:], utc, AL.mult)
                    nq3 = sb.tile([H, CH], BF16, tag="q3")
                    nc.gpsimd.tensor_tensor(nq3[:], sp_t[:], uxxc, AL.mult)
                    ax_t, at_t, q3_t, q2_t = nax, nat, nq3, nq2

                # L4: u = W4.T @ stream (M=1).  Matmul PSUM outputs may only
                # start at partition 0/32/64: pack u/ux/ut at 0/32/64 col 0,
                # uxx (2-matmul accum) at partition 0 of the second bank.
                uo = psu.tile([65, 2, CH], F32, tag="uo")
                for j, rhs in enumerate((a_t, ax_t, at_t)):
                    nc.tensor.matmul(uo[32 * j:32 * j + 1, 0, :], w4_sb[:], rhs[:],
                                     start=True, stop=True)
                nc.tensor.matmul(uo[0:1, 1, :], w4_sb[:], q3_t[:], start=True, stop=False)
                nc.tensor.matmul(uo[0:1, 1, :], w4_sb[:], q2_t[:], start=False, stop=True)
                ue = out1.tile([65, 2, CH], F32, tag="ue")
                nc.scalar.activation(ue[:], uo[:], AF.Copy)
                for j in range(3):
                    nc.sync.dma_start(stage[j][c:c + 1, :], ue[32 * j:32 * j + 1, 0, :])
                nc.sync.dma_start(stage[3][c:c + 1, :], ue[0:1, 1, :])

            # ---- phase 2: residual + reduction ----
            sv = []
            for i in range(4):
                t = ph2.tile([128, F2], F32, tag=f"pu{i}")
                nc.sync.dma_start(t[:], stage[i].rearrange("a b -> (a b)").rearrange("(p f) -> p f", p=128))
                sv.append(t)
            u_v, ux_v, ut_v, uxx_v = sv
            cf = []
            for k in range(5):
                t = ph2.tile([128, F2], F32, tag=f"pcf{k}")
                nc.sync.dma_start(t[:], coef_in[k].rearrange("(p f) -> p f", p=128))
                cf.append(t)
            cA, cB, cC1, cC2, cT = cf

            r = ph2.tile([128, F2], F32)
            m = ph2.tile([128, F2], F32)
            nc.vector.tensor_tensor(r[:], ut_v[:], cA[:], AL.mult)
            nc.vector.tensor_tensor(m[:], uxx_v[:], cB[:], AL.mult)
            nc.vector.tensor_tensor(r[:], r[:], m[:], AL.add)
            nc.vector.tensor_tensor(m[:], ux_v[:], cC1[:], AL.mult)
            nc.vector.tensor_tensor(r[:], r[:], m[:], AL.add)
            nc.vector.tensor_tensor(m[:], u_v[:], cC2[:], AL.mult)
            nc.vector.tensor_tensor(r[:], r[:], m[:], AL.add)
            nc.vector.tensor_tensor(r[:], r[:], cT[:], AL.subtract)

            rsq = ph2.tile([128, F2], F32)
            racc = ph2.tile([128, 1], F32)
            nc.vector.scalar_tensor_tensor(rsq[:], r[:], 1.0, r[:], AL.mult, AL.mult,
                                           accum_out=racc[:])
            lps = psu.tile([1, 1], F32, tag="uo")
            nc.tensor.matmul(lps[:], racc[:], ones_sb[:], start=True, stop=True)
            lsb = ph2.tile([1, 1], F32)
            nc.vector.tensor_copy(lsb[:], lps[:])
            nc.sync.dma_start(part_out[:], lsb[:])

    return nc


# ---------------------------------------------------------------------------
# Host-side sharding + coefficient prep
# ---------------------------------------------------------------------------
def _host_inputs(inputs):
    x_col = np.asarray(inputs["x_col"], np.float64).reshape(-1)
    t_col = np.asarray(inputs["t_col"], np.float64).reshape(-1)
    x_ic = np.asarray(inputs["x_ic"], np.float64).reshape(-1)
    t_ic = np.asarray(inputs["t_ic"], np.float64).reshape(-1)
    x_bl = np.asarray(inputs["x_bc_left"], np.float64).reshape(-1)
    x_br = np.asarray(inputs["x_bc_right"], np.float64).reshape(-1)
    t_bc = np.asarray(inputs["t_bc"], np.float64).reshape(-1)
    W0 = np.asarray(inputs["W0"], np.float32)
    W4 = np.asarray(inputs["W4"], np.float32)
    b4 = float(np.asarray(inputs["b4"]).reshape(-1)[0])
    D = float(np.asarray(inputs["D"]))
    chi = float(np.asarray(inputs["chi"]))

    def S(x):
        return np.exp(-((x - 0.7) ** 2) / 0.02)

    def Sp(x):
        return -(x - 0.7) / 0.01 * S(x)

    def Spp(x):
        return S(x) * (((x - 0.7) ** 2) / 1.0e-4 - 100.0)

    def icf(x):
        return 0.1 + 0.05 * np.exp(-((x - 0.3) ** 2) / 0.01)

    swc = (1.0 / N_COL) ** 0.5
    swi = (1.0 / N_IC) ** 0.5
    swb = (1.0 / N_BC) ** 0.5

    bf16 = ml_dtypes.bfloat16
    pc = np.zeros((H, 8), np.float32)
    for i, k in enumerate(("b0", "b1", "b2", "b3")):
        pc[:, i] = np.asarray(inputs[k], np.float32)
    pc[:, 4] = W0[0]
    pc[:, 5] = W0[1]
    pc[:, 6] = W0[0] ** 2
    w0_16 = W0.astype(bf16)
    w123_16 = np.stack([np.asarray(inputs[k], np.float32) for k in ("W1", "W2", "W3")]).astype(bf16)
    w4_16 = W4.astype(bf16)

    in_maps = []
    for c in range(N_CORES):
        xs = np.full(NPTS, 0.5, np.float64)
        ts = np.full(NPTS, 0.5, np.float64)
        A = np.zeros(NPTS, np.float64)
        B = np.zeros(NPTS, np.float64)
        C1 = np.zeros(NPTS, np.float64)
        C2 = np.zeros(NPTS, np.float64)
        TG = np.zeros(NPTS, np.float64)

        o = 0
        sl = slice(c * COLC, (c + 1) * COLC)
        xs[o:o + COLC] = x_col[sl]
        ts[o:o + COLC] = t_col[sl]
        A[o:o + COLC] = swc
        B[o:o + COLC] = -D * swc
        C1[o:o + COLC] = chi * Sp(x_col[sl]) * swc
        C2[o:o + COLC] = chi * Spp(x_col[sl]) * swc
        o += COLC
        sl = slice(c * ICC, (c + 1) * ICC)
        xs[o:o + ICC] = x_ic[sl]
        ts[o:o + ICC] = t_ic[sl]
        C2[o:o + ICC] = swi
        TG[o:o + ICC] = swi * icf(x_ic[sl])
        o += ICC
        sl = slice(c * BCC, (c + 1) * BCC)
        xs[o:o + BCC] = x_bl[sl]
        ts[o:o + BCC] = t_bc[sl]
        C1[o:o + BCC] = swb
        o += BCC
        xs[o:o + BCC] = x_br[sl]
        ts[o:o + BCC] = t_bc[sl]
        C1[o:o + BCC] = swb
        o += BCC

        TG = TG - C2 * b4  # fold the final-layer bias into the target
        xt = np.stack([xs, ts]).astype(bf16)
        coef = np.stack([A, B, C1, C2, TG]).astype(np.float32)
        in_maps.append({
            "xt": xt, "coef": coef,
            "w0": w0_16, "w123": w123_16, "w4": w4_16, "pc": pc,
        })
    return in_maps


_CACHE = {}


def _get_nc():
    if "nc" not in _CACHE:
        _CACHE["nc"] = build_program()
    return _CACHE["nc"]


def run(inputs, trace=False):
    nc = _get_nc()
    in_maps = _host_inputs(inputs)
    res = run_bass_kernel_spmd(nc, in_maps, list(range(N_CORES)), trace=trace)
    total = 0.0
    for i in range(N_CORES):
        total += float(res.results[i]["partial"][0, 0])
    return np.float32(total), res


def kernel(**inputs):
    loss, _ = run(inputs)
    return np.asarray(loss, np.float32)


# revision 12
# speedup vs baseline: 1.6622x; 1.6622x over previous
"""Trainium2 Bass kernel for the ChemotaxisPINN loss.

Computes loss = mean_col((u_t - D*u_xx + chi*(u_x*S'(x) + u*S''(x)))^2)
             + mean_ic((u - ic(x))^2) + mean_bc(u_x(0,t)^2) + mean_bc(u_x(1,t)^2)
for a 5-layer SiLU MLP u(x,t), via forward-mode AD on device.

Strategy: pure data parallel over 8 NeuronCores. Each core gets a shard of
collocation/ic/bc points (17134 valid, padded to 17408 = 34 chunks x 512).
On device, per 512-point chunk, the MLP is evaluated with 4 propagated
streams (value, d/dx, d/dt, d2/dx2) in a feature-major layout
[128 features x points]; per-point residual coefficients are precomputed on
the host and folded so that loss = sum over all points of r^2 with
r = A*u_t + B*u_xx + C1*u_x + C2*u - T.  Each core returns a partial sum;
the host adds the 8 partials.

Perf layout (v2): bf16 matmuls/activations (fp32 PSUM accum), a fused
custom-DVE op for the silu'/silu'' chains, the three tangent preactivations
packed in one 3-bank PSUM tile evacuated by a single scalar ACT, and the
four scalar outputs packed in one 2-bank PSUM tile evacuated by one ACT.
Elementwise work is spread over scalar/vector/gpsimd.

silu-family identities (z = preactivation + bias, s = sigmoid(z)):
  a   = z * s                 = silu(z)
  sp  = a + s*(1-a)           = silu'(z)   (one fused DVE op)
  spp = s + sp*(1-2s)         = silu''(z)  (one fused DVE op)
"""

import numpy as np

import orjson
import concourse.bass as bass
import concourse.tile as tile
from concourse import mybir
from concourse.bass_utils import run_bass_kernel_spmd

F32 = mybir.dt.float32
F16 = mybir.dt.float16
AL = mybir.AluOpType
AF = mybir.ActivationFunctionType

N_CORES = 8
H = 128
N_COL, N_IC, N_BC = 131072, 2000, 2000
COLC = N_COL // N_CORES     # 16384
ICC = N_IC // N_CORES       # 250
BCC = N_BC // N_CORES       # 250
NVALID = COLC + ICC + 2 * BCC   # 17134
CH = 512                     # points per chunk
NCHUNK = (NVALID + CH - 1) // CH  # 34
NPTS = NCHUNK * CH           # 17408
F2 = NPTS // 128             # 136 (phase-2 free dim)

# ---------------------------------------------------------------------------
# BIR fix: this walrus build accepts at most ONE sem wait per instruction,
# while Tile attaches several.  Split extras onto single-wait NoOps.
# ---------------------------------------------------------------------------
_orig_to_json_bytes = bass.Bass.to_json_bytes


def _split_multiwait(m):
    for fn in m.get("functions", []):
        for blk in fn.get("blocks", []):
            insts = blk.get("instructions", [])
            out = []
            changed = False
            ctr = 0
            for inst in insts:
                si = inst.get("sync_info")
                waits = (si or {}).get("on_wait") or []
                if len(waits) > 1:
                    changed = True
                    for w in waits[:-1]:
                        ctr += 1
                        out.append({
                            "engine": inst["engine"],
                            "ins": [],
                            "outs": [],
                            "name": f"I-mws-{ctr}-{inst.get('name', '')}",
                            "opcode": "NoOp",
                            "sync_info": {"on_wait": [w], "on_update": []},
                            "debug": inst.get("debug", 0),
                        })
                    si["on_wait"] = waits[-1:]
                out.append(inst)
            if changed:
                blk["instructions"] = out
    return m


def _patched_to_json_bytes(self):
    return orjson.dumps(_split_multiwait(orjson.loads(_orig_to_json_bytes(self))))


bass.Bass.to_json_bytes = _patched_to_json_bytes


# ---------------------------------------------------------------------------
# Device program
# ---------------------------------------------------------------------------
def build_program():
    nc = bass.Bass("TRN2", target_bir_lowering=False, debug=False)

    xt_in = nc.declare_dram_parameter("xt", [2, NPTS], F16, isOutput=False)
    coef_in = nc.declare_dram_parameter("coef", [5, NPTS], F32, isOutput=False)
    w0_in = nc.declare_dram_parameter("w0", [2, H], F16, isOutput=False)
    w123_in = nc.declare_dram_parameter("w123", [3, H, H], F16, isOutput=False)
    w4_in = nc.declare_dram_parameter("w4", [H, 1], F16, isOutput=False)
    pc_in = nc.declare_dram_parameter("pc", [H, 8], F32, isOutput=False)
    part_out = nc.declare_dram_parameter("partial", [1, 1], F32, isOutput=True)

    # DRAM staging for per-point u, ux, ut, uxx (stream-major, chunk rows)
    stage = nc.dram_tensor("stage", [4, NCHUNK, CH], F32)

    with tile.TileContext(nc) as tc:
        with (
            tc.tile_pool(name="consts", bufs=1) as cn,
            tc.tile_pool(name="sb", bufs=3) as sb,
            tc.tile_pool(name="out1", bufs=3) as out1,
            tc.tile_pool(name="ph2", bufs=1) as ph2,
            tc.tile_pool(name="psA", bufs=2, space="PSUM") as psA,
            tc.tile_pool(name="psB", bufs=1, space="PSUM") as psB,
            tc.tile_pool(name="psu", bufs=1, space="PSUM") as psu,
        ):
            # ---- constants ----
            w0_sb = cn.tile([2, H], F16)
            nc.sync.dma_start(w0_sb[:], w0_in[:])
            w123_sb = cn.tile([H, 3, H], F16)
            for l in range(3):
                nc.sync.dma_start(w123_sb[:, l, :], w123_in[l])
            w4_sb = cn.tile([H, 1], F16)
            nc.sync.dma_start(w4_sb[:], w4_in[:])
            pc_sb = cn.tile([H, 8], F32)
            nc.sync.dma_start(pc_sb[:], pc_in[:])
            ones_sb = cn.tile([H, 1], F32)
            nc.vector.memset(ones_sb[:], 1.0)
            # pc columns: 0..3 = b0..b3, 4 = W0[0], 5 = W0[1], 6 = W0[0]^2
            b_ap = [pc_sb[:, i:i + 1] for i in range(4)]
            w0x_ap = pc_sb[:, 4:5]
            w0t_ap = pc_sb[:, 5:6]
            w0x2_ap = pc_sb[:, 6:7]

            # ---- phase 1: per-chunk MLP + tangents ----
            for c in range(NCHUNK):
                xt_sb = sb.tile([2, CH], F16, tag="xt")
                nc.sync.dma_start(xt_sb[:], xt_in[:, c * CH:(c + 1) * CH])

                # L0: z0 = W0.T @ [x;t]  (K=2)
                z0 = psA.tile([H, CH], F32, tag="z")
                nc.tensor.matmul(z0[:], w0_sb[:], xt_sb[:], start=True, stop=True)

                s_t = sb.tile([H, CH], F16, tag="s")
                nc.scalar.activation(s_t[:], z0[:], AF.Sigmoid, bias=b_ap[0])
                a_t = sb.tile([H, CH], F16, tag="a")
                nc.vector.scalar_tensor_tensor(a_t[:], z0[:], b_ap[0], s_t[:], AL.add, AL.mult)
                t_t = sb.tile([H, CH], F16, tag="t")
                nc.vector.scalar_tensor_tensor(t_t[:], a_t[:], -1.0, s_t[:], AL.add, AL.mult)
                sp_t = sb.tile([H, CH], F16, tag="sp")
                nc.vector.tensor_tensor(sp_t[:], a_t[:], t_t[:], AL.subtract)
                q_t = sb.tile([H, CH], F16, tag="q")
                nc.vector.scalar_tensor_tensor(q_t[:], s_t[:], -0.5, sp_t[:], AL.add, AL.mult)
                spp_t = sb.tile([H, CH], F16, tag="spp")
                nc.vector.scalar_tensor_tensor(spp_t[:], q_t[:], -2.0, s_t[:], AL.mult, AL.add)

                # L0 tangent seeds (per-partition constants)
                ax_t = sb.tile([H, CH], F16, tag="ax")
                nc.scalar.activation(ax_t[:], sp_t[:], AF.Identity, scale=w0x_ap)
                at_t = sb.tile([H, CH], F16, tag="at")
                nc.vector.tensor_scalar(at_t[:], sp_t[:], w0t_ap, None, AL.mult)
                q3_t = sb.tile([H, CH], F16, tag="q3")
                nc.vector.tensor_scalar(q3_t[:], spp_t[:], w0x2_ap, None, AL.mult)
                q2_t = None  # L0 xx-stream has a single component

                # hidden layers 1..3
                for l in range(3):
                    W = w123_sb[:, l, :]
                    zv = psA.tile([H, CH], F32, tag="z")
                    nc.tensor.matmul(zv[:], W, a_t[:], start=True, stop=True)
                    zt3 = psB.tile([H, 3, CH], F32, tag="zt3")
                    nc.tensor.matmul(zt3[:, 0, :], W, ax_t[:], start=True, stop=True)
                    nc.tensor.matmul(zt3[:, 1, :], W, at_t[:], start=True, stop=True)
                    if q2_t is None:
                        nc.tensor.matmul(zt3[:, 2, :], W, q3_t[:], start=True, stop=True)
                    else:
                        nc.tensor.matmul(zt3[:, 2, :], W, q3_t[:], start=True, stop=False)
                        nc.tensor.matmul(zt3[:, 2, :], W, q2_t[:], start=False, stop=True)

                    b = b_ap[l + 1]
                    s_t = sb.tile([H, CH], F16, tag="s")
                    nc.scalar.activation(s_t[:], zv[:], AF.Sigmoid, bias=b)
                    a_t = sb.tile([H, CH], F16, tag="a")
                    nc.vector.scalar_tensor_tensor(a_t[:], zv[:], b, s_t[:], AL.add, AL.mult)
                    t_t = sb.tile([H, CH], F16, tag="t")
                    nc.vector.scalar_tensor_tensor(t_t[:], a_t[:], -1.0, s_t[:], AL.add, AL.mult)
                    sp_t = sb.tile([H, CH], F16, tag="sp")
                    nc.vector.tensor_tensor(sp_t[:], a_t[:], t_t[:], AL.subtract)
                    q_t = sb.tile([H, CH], F16, tag="q")
                    nc.vector.scalar_tensor_tensor(q_t[:], s_t[:], -0.5, sp_t[:], AL.add, AL.mult)
                    spp_t = sb.tile([H, CH], F16, tag="spp")
                    nc.vector.scalar_tensor_tensor(spp_t[:], q_t[:], -2.0, s_t[:], AL.mult, AL.add)

                    # single evacuation of the 3 tangent preactivations
                    uxtc = sb.tile([H, 3, CH], F16, tag="uxtc")
                    nc.scalar.activation(uxtc[:], zt3[:], AF.Copy)
                    uxc = uxtc[:, 0, :]
                    utc = uxtc[:, 1, :]
                    uxxc = uxtc[:, 2, :]
                    zx2_t = sb.tile([H, CH], F16, tag="zx2")
                    nc.scalar.activation(zx2_t[:], uxc, AF.Square)

                    nax = sb.tile([H, CH], F16, tag="ax")
                    nc.vector.tensor_tensor(nax[:], sp_t[:], uxc, AL.mult)
                    nq2 = sb.tile([H, CH], F16, tag="q2")
                    nc.vector.tensor_tensor(nq2[:], spp_t[:], zx2_t[:], AL.mult)
                    nat = sb.tile([H, CH], F16, tag="at")
                    nc.gpsimd.tensor_tensor(nat[:], sp_t[:], utc, AL.mult)
                    nq3 = sb.tile([H, CH], F16, tag="q3")
                    nc.gpsimd.tensor_tensor(nq3[:], sp_t[:], uxxc, AL.mult)
                    ax_t, at_t, q3_t, q2_t = nax, nat, nq3, nq2

                # L4: u = W4.T @ stream (M=1).  Matmul PSUM outputs may only
                # start at partition 0/32/64: pack u/ux/ut at 0/32/64 col 0,
                # uxx (2-matmul accum) at partition 0 of the second bank.
                uo = psu.tile([65, 2, CH], F32, tag="uo")
                for j, rhs in enumerate((a_t, ax_t, at_t)):
                    nc.tensor.matmul(uo[32 * j:32 * j + 1, 0, :], w4_sb[:], rhs[:],
                                     start=True, stop=True)
                nc.tensor.matmul(uo[0:1, 1, :], w4_sb[:], q3_t[:], start=True, stop=False)
                nc.tensor.matmul(uo[0:1, 1, :], w4_sb[:], q2_t[:], start=False, stop=True)
                ue = out1.tile([65, 2, CH], F32, tag="ue")
                nc.scalar.activation(ue[:], uo[:], AF.Copy)
                for j in range(3):
                    nc.sync.dma_start(stage[j][c:c + 1, :], ue[32 * j:32 * j + 1, 0, :])
                nc.sync.dma_start(stage[3][c:c + 1, :], ue[0:1, 1, :])

            # ---- phase 2: residual + reduction ----
            sv = []
            for i in range(4):
                t = ph2.tile([128, F2], F32, tag=f"pu{i}")
                nc.sync.dma_start(t[:], stage[i].rearrange("a b -> (a b)").rearrange("(p f) -> p f", p=128))
                sv.append(t)
            u_v, ux_v, ut_v, uxx_v = sv
            cf = []
            for k in range(5):
                t = ph2.tile([128, F2], F32, tag=f"pcf{k}")
                nc.sync.dma_start(t[:], coef_in[k].rearrange("(p f) -> p f", p=128))
                cf.append(t)
            cA, cB, cC1, cC2, cT = cf

            r = ph2.tile([128, F2], F32)
            m = ph2.tile([128, F2], F32)
            nc.vector.tensor_tensor(r[:], ut_v[:], cA[:], AL.mult)
            nc.vector.tensor_tensor(m[:], uxx_v[:], cB[:], AL.mult)
            nc.vector.tensor_tensor(r[:], r[:], m[:], AL.add)
            nc.vector.tensor_tensor(m[:], ux_v[:], cC1[:], AL.mult)
            nc.vector.tensor_tensor(r[:], r[:], m[:], AL.add)
            nc.vector.tensor_tensor(m[:], u_v[:], cC2[:], AL.mult)
            nc.vector.tensor_tensor(r[:], r[:], m[:], AL.add)
            nc.vector.tensor_tensor(r[:], r[:], cT[:], AL.subtract)

            rsq = ph2.tile([128, F2], F32)
            racc = ph2.tile([128, 1], F32)
            nc.vector.scalar_tensor_tensor(rsq[:], r[:], 1.0, r[:], AL.mult, AL.mult,
                                           accum_out=racc[:])
            lps = psu.tile([1, 1], F32, tag="uo")
            nc.tensor.matmul(lps[:], racc[:], ones_sb[:], start=True, stop=True)
            lsb = ph2.tile([1, 1], F32)
            nc.vector.tensor_copy(lsb[:], lps[:])
            nc.sync.dma_start(part_out[:], lsb[:])

    return nc


# ---------------------------------------------------------------------------
# Host-side sharding + coefficient prep
# ---------------------------------------------------------------------------
def _host_inputs(inputs):
    x_col = np.asarray(inputs["x_col"], np.float64).reshape(-1)
    t_col = np.asarray(inputs["t_col"], np.float64).reshape(-1)
    x_ic = np.asarray(inputs["x_ic"], np.float64).reshape(-1)
    t_ic = np.asarray(inputs["t_ic"], np.float64).reshape(-1)
    x_bl = np.asarray(inputs["x_bc_left"], np.float64).reshape(-1)
    x_br = np.asarray(inputs["x_bc_right"], np.float64).reshape(-1)
    t_bc = np.asarray(inputs["t_bc"], np.float64).reshape(-1)
    W0 = np.asarray(inputs["W0"], np.float32)
    W4 = np.asarray(inputs["W4"], np.float32)
    b4 = float(np.asarray(inputs["b4"]).reshape(-1)[0])
    D = float(np.asarray(inputs["D"]))
    chi = float(np.asarray(inputs["chi"]))

    def S(x):
        return np.exp(-((x - 0.7) ** 2) / 0.02)

    def Sp(x):
        return -(x - 0.7) / 0.01 * S(x)

    def Spp(x):
        return S(x) * (((x - 0.7) ** 2) / 1.0e-4 - 100.0)

    def icf(x):
        return 0.1 + 0.05 * np.exp(-((x - 0.3) ** 2) / 0.01)

    swc = (1.0 / N_COL) ** 0.5
    swi = (1.0 / N_IC) ** 0.5
    swb = (1.0 / N_BC) ** 0.5

    pc = np.zeros((H, 8), np.float32)
    for i, k in enumerate(("b0", "b1", "b2", "b3")):
        pc[:, i] = np.asarray(inputs[k], np.float32)
    pc[:, 4] = W0[0]
    pc[:, 5] = W0[1]
    pc[:, 6] = W0[0] ** 2
    w0_16 = W0.astype(np.float16)
    w123_16 = np.stack([np.asarray(inputs[k], np.float32) for k in ("W1", "W2", "W3")]).astype(np.float16)
    w4_16 = W4.astype(np.float16)

    in_maps = []
    for c in range(N_CORES):
        xs = np.full(NPTS, 0.5, np.float64)
        ts = np.full(NPTS, 0.5, np.float64)
        A = np.zeros(NPTS, np.float64)
        B = np.zeros(NPTS, np.float64)
        C1 = np.zeros(NPTS, np.float64)
        C2 = np.zeros(NPTS, np.float64)
        TG = np.zeros(NPTS, np.float64)

        o = 0
        sl = slice(c * COLC, (c + 1) * COLC)
        xs[o:o + COLC] = x_col[sl]
        ts[o:o + COLC] = t_col[sl]
        A[o:o + COLC] = swc
        B[o:o + COLC] = -D * swc
        C1[o:o + COLC] = chi * Sp(x_col[sl]) * swc
        C2[o:o + COLC] = chi * Spp(x_col[sl]) * swc
        o += COLC
        sl = slice(c * ICC, (c + 1) * ICC)
        xs[o:o + ICC] = x_ic[sl]
        ts[o:o + ICC] = t_ic[sl]
        C2[o:o + ICC] = swi
        TG[o:o + ICC] = swi * icf(x_ic[sl])
        o += ICC
        sl = slice(c * BCC, (c + 1) * BCC)
        xs[o:o + BCC] = x_bl[sl]
        ts[o:o + BCC] = t_bc[sl]
        C1[o:o + BCC] = swb
        o += BCC
        xs[o:o + BCC] = x_br[sl]
        ts[o:o + BCC] = t_bc[sl]
        C1[o:o + BCC] = swb
        o += BCC

        TG = TG - C2 * b4  # fold the final-layer bias into the target
        xt = np.stack([xs, ts]).astype(np.float16)
        coef = np.stack([A, B, C1, C2, TG]).astype(np.float32)
        in_maps.append({
            "xt": xt, "coef": coef,
            "w0": w0_16, "w123": w123_16, "w4": w4_16, "pc": pc,
        })
    return in_maps


_CACHE = {}


def _get_nc():
    if "nc" not in _CACHE:
        _CACHE["nc"] = build_program()
    return _CACHE["nc"]


def run(inputs, trace=False):
    nc = _get_nc()
    in_maps = _host_inputs(inputs)
    res = run_bass_kernel_spmd(nc, in_maps, list(range(N_CORES)), trace=trace)
    total = 0.0
    for i in range(N_CORES):
        total += float(res.results[i]["partial"][0, 0])
    return np.float32(total), res


def kernel(**inputs):
    loss, _ = run(inputs)
    return np.asarray(loss, np.float32)
